# revision 32
# baseline (speedup 1.0000x reference)
"""DeepKalman (prior-rollout + LSTM recognition + generator + KL/recon ELBO)
Trainium2 Bass/Tile kernel, data-parallel over batch across 8 NeuronCores.

Layout strategy: everything transposed — feature dims on SBUF partitions,
(t, b) flattened on the free dim. The LSTM runs as a 512-step serial chain
of 128x128 bf16 matmuls; sigmoid is computed as scaled tanh so the whole
chain needs only the `exp_and_others` ACT table set; all softplus/log work
is deferred to a single post-chain phase using `natural_log_exp_and_others`
(exactly one activation-table switch in the whole kernel).

Scaling folds (exact powers of two, so they cost nothing numerically):
  - sigmoid(x) = 0.5*(1 + tanh(x/2)); one ACT op computes tanh(z/2) for all
    four gates at once, with the g-gate's weights pre-multiplied by 2 so the
    same op yields tanh(g) there.
  - The DVE cell update produces Hs = 2*h; R and Wr are pre-halved.
  - Cell state is stored as 2*c; tanh(c) is ACT tanh with scale=0.5.
"""

import numpy as np

B_GLOBAL = 256
NCORES = 8
BL = B_GLOBAL // NCORES          # batch per core = 32
T_FULL = 512
S = 64
O = 128
H = 128                          # LSTM hidden = 2*S
LOG2PI = float(np.log(2.0 * np.pi))
GSLOT = [0, 1, 3, 2]             # slot order [i, f, o, g] <- original chunks


def _build(T=T_FULL, CH=32):
    import os as _os
    ABL = _os.environ.get("KABL", "")   # dev-only ablation switches
    import concourse.bass as bass
    import concourse.mybir as mybir
    from concourse import bacc
    from concourse.tile import TileContext

    f32 = mybir.dt.float32
    bf16 = mybir.dt.bfloat16
    FT = mybir.ActivationFunctionType
    ALU = mybir.AluOpType

    NCHUNK = T // CH
    TB = T * BL                  # free-dim length of the (t, b) stashes

    nc = bacc.Bacc("TRN2", target_bir_lowering=False)

    obs = nc.dram_tensor("obs", [BL, T, O], f32, kind="ExternalInput")
    im = nc.dram_tensor("initial_mean", [BL, S], f32, kind="ExternalInput")
    Wf_m = nc.dram_tensor("Wf_m", [S, S], f32, kind="ExternalInput")
    bf_m = nc.dram_tensor("bf_m", [S], f32, kind="ExternalInput")
    Wf_s = nc.dram_tensor("Wf_s", [S, S], f32, kind="ExternalInput")
    bf_s = nc.dram_tensor("bf_s", [S], f32, kind="ExternalInput")
    Wg_m = nc.dram_tensor("Wg_m", [S, O], f32, kind="ExternalInput")
    bg_m = nc.dram_tensor("bg_m", [O], f32, kind="ExternalInput")
    Wg_s = nc.dram_tensor("Wg_s", [S, O], f32, kind="ExternalInput")
    bg_s = nc.dram_tensor("bg_s", [O], f32, kind="ExternalInput")
    K_l = nc.dram_tensor("K_lstm", [O, 4 * H], f32, kind="ExternalInput")
    R_l = nc.dram_tensor("R_lstm", [H, 4 * H], f32, kind="ExternalInput")
    b_l = nc.dram_tensor("b_lstm", [4 * H], f32, kind="ExternalInput")
    Wr_m = nc.dram_tensor("Wr_m", [H, S], f32, kind="ExternalInput")
    br_m = nc.dram_tensor("br_m", [S], f32, kind="ExternalInput")
    Wr_s = nc.dram_tensor("Wr_s", [H, S], f32, kind="ExternalInput")
    br_s = nc.dram_tensor("br_s", [S], f32, kind="ExternalInput")
    out_d = nc.dram_tensor("out", [1, 8], f32, kind="ExternalOutput")

    with TileContext(nc) as tc:
        with (
            tc.tile_pool(name="persist", bufs=1) as P,
            tc.tile_pool(name="stage", bufs=1) as STG,
        ):
            # ---------------- persistent SBUF tensors ----------------
            obsT = P.tile([128, TB], bf16)       # obs^T, col = t*32+b
            Hs = P.tile([128, TB], bf16)         # 2*h stash
            Pst = P.tile([128, TB], bf16)        # prior stash: [m | sn]
            K_bf = P.tile([128, 4 * H], bf16)
            R_bf = P.tile([128, 4 * H], bf16)    # pre-scaled R/2 (g slot: R_g)
            Wf_bf = P.tile([S, 2 * S], bf16)     # [Wf_m | Wf_s]
            Wr_bf = P.tile([128, 2 * S], bf16)   # [Wr_m | Wr_s] / 2
            Wgm_bf = P.tile([S, O], bf16)
            Wgs_bf = P.tile([S, O], bf16)
            b_bf = P.tile([1, 4 * H], bf16)
            brS_bf = P.tile([1, 2 * S], bf16)
            bgm_bf = P.tile([1, O], bf16)
            bgs_bf = P.tile([1, O], bf16)
            ones_bf = P.tile([1, 512], bf16)
            idn_f32 = P.tile([128, 128], f32)
            idn_bf = P.tile([128, 128], bf16)
            pbias = P.tile([128, 1], f32)        # [bf_m | bf_s]
            tzA = P.tile([128, 5 * BL], bf16)    # [i f o g | C(next)]
            tzB = P.tile([128, 5 * BL], bf16)
            one_c = P.tile([128, 1], bf16)
            half_c = P.tile([128, 1], bf16)
            minit = P.tile([S, BL], bf16)
            CMB = P.tile([128, TB], bf16)    # lo: pm (gen rhs); hi: exp(sppre)
            NBLK = TB // 512
            acc = P.tile([128, 5 * NBLK], f32)
            c0 = P.tile([128, BL], f32)
            ones32 = P.tile([128, BL], f32)

            def accsl(psl, kind, blk):
                return acc[psl, kind * NBLK + blk: kind * NBLK + blk + 1]

            # ---------------- init: load + cast weights ----------------
            def cast(dst, src_sb, scale=1.0):
                if scale == 1.0:
                    nc.vector.tensor_copy(dst, src_sb)
                else:
                    nc.scalar.mul(dst, src_sb, scale)

            k_st = STG.tile([128, 4 * H], f32)
            nc.sync.dma_start(out=k_st, in_=K_l.ap())
            r_st = STG.tile([128, 4 * H], f32)
            nc.sync.dma_start(out=r_st, in_=R_l.ap())
            for s in range(4):
                g = GSLOT[s]
                ksc = 2.0 if s == 3 else 1.0     # g-gate weights doubled
                rsc = 1.0 if s == 3 else 0.5     # and R pre-halved (Hs = 2h)
                cast(K_bf[:, s * H:(s + 1) * H], k_st[:, g * H:(g + 1) * H], ksc)
                cast(R_bf[:, s * H:(s + 1) * H], r_st[:, g * H:(g + 1) * H], rsc)
            b_st = STG.tile([1, 4 * H], f32)
            nc.sync.dma_start(out=b_st, in_=b_l.ap().rearrange("(o n) -> o n", o=1))
            for s in range(4):
                g = GSLOT[s]
                cast(b_bf[:, s * H:(s + 1) * H], b_st[:, g * H:(g + 1) * H],
                     2.0 if s == 3 else 1.0)

            wf_st = STG.tile([S, 2 * S], f32)
            nc.sync.dma_start(out=wf_st[:, 0:S], in_=Wf_m.ap())
            nc.sync.dma_start(out=wf_st[:, S:2 * S], in_=Wf_s.ap())
            cast(Wf_bf, wf_st)
            wr_st = STG.tile([128, 2 * S], f32)
            nc.sync.dma_start(out=wr_st[:, 0:S], in_=Wr_m.ap())
            nc.sync.dma_start(out=wr_st[:, S:2 * S], in_=Wr_s.ap())
            cast(Wr_bf, wr_st, 0.5)
            wg_st = STG.tile([S, 2 * O], f32)
            nc.sync.dma_start(out=wg_st[:, 0:O], in_=Wg_m.ap())
            nc.sync.dma_start(out=wg_st[:, O:2 * O], in_=Wg_s.ap())
            cast(Wgm_bf, wg_st[:, 0:O])
            cast(Wgs_bf, wg_st[:, O:2 * O])
            bsm_st = STG.tile([1, 2 * S + 2 * O], f32)
            nc.sync.dma_start(out=bsm_st[:, 0:S], in_=br_m.ap().rearrange("(o n) -> o n", o=1))
            nc.sync.dma_start(out=bsm_st[:, S:2 * S], in_=br_s.ap().rearrange("(o n) -> o n", o=1))
            nc.sync.dma_start(out=bsm_st[:, 2 * S:2 * S + O], in_=bg_m.ap().rearrange("(o n) -> o n", o=1))
            nc.sync.dma_start(out=bsm_st[:, 2 * S + O:], in_=bg_s.ap().rearrange("(o n) -> o n", o=1))
            cast(brS_bf, bsm_st[:, 0:2 * S])
            cast(bgm_bf, bsm_st[:, 2 * S:2 * S + O])
            cast(bgs_bf, bsm_st[:, 2 * S + O:])
            nc.sync.dma_start(out=pbias[0:S, :], in_=bf_m.ap().rearrange("(n o) -> n o", o=1))
            nc.sync.dma_start(out=pbias[S:2 * S, :], in_=bf_s.ap().rearrange("(n o) -> n o", o=1))

            from concourse.masks import make_identity
            make_identity(nc, idn_f32)
            nc.vector.tensor_copy(idn_bf, idn_f32)
            nc.vector.memset(ones_bf, 1.0)
            nc.vector.memset(acc, 0.0)
            nc.vector.memset(c0, 0.0)
            nc.vector.memset(tzA[:, 4 * BL:5 * BL], 0.0)
            nc.vector.memset(one_c, 1.0)
            nc.vector.memset(half_c, 0.5)
            nc.vector.memset(ones32, 1.0)

            im_st = STG.tile([BL, S], f32)
            nc.sync.dma_start(out=im_st, in_=im.ap())

            obs_v = obs.ap().rearrange("b (ck j ti) o -> ck j b ti o",
                                  ck=NCHUNK, j=CH // 4, ti=4)

            lo = slice(0, S)
            hi = slice(S, 2 * S)
            with (
                tc.tile_pool(name="zps", bufs=2, space="PSUM") as ZPS,
                tc.tile_pool(name="pps", bufs=1, space="PSUM") as PPS,
                tc.tile_pool(name="xps", bufs=2, space="PSUM") as XPS,
                tc.tile_pool(name="tps", bufs=1, space="PSUM") as TPS,
                tc.tile_pool(name="fps", bufs=1, space="PSUM") as FPS,
                tc.tile_pool(name="ring", bufs=2) as RING,
                tc.tile_pool(name="onat", bufs=2) as ONAT,
                tc.tile_pool(name="cw", bufs=2) as CW,
            ):
                # initial_mean^T -> minit (bf16)
                imt_ps = TPS.tile([128, 128], f32, tag="tp")
                nc.tensor.transpose(imt_ps[0:S, 0:BL], im_st, idn_f32[0:BL, 0:BL])
                nc.vector.tensor_copy(minit, imt_ps[0:S, 0:BL])

                zx_tiles = {}

                def phase_a(k):
                    """obs chunk k: DMA in, PE-transpose into obsT, then
                    Zx = obs^T @ K + b into the bf16 ring buffer."""
                    onat = ONAT.tile([128, CH // 4, 128], f32, tag="on")
                    for j in range(CH // 4):
                        nc.sync.dma_start(out=onat[:, j, :], in_=obs_v[k, j])
                    base = k * CH * BL
                    for j in range(CH // 4):
                        tp = TPS.tile([128, 128], f32, tag="tp")
                        nc.tensor.transpose(tp, onat[:, j, :], idn_f32)
                        dst = obsT[:, base + j * 128: base + (j + 1) * 128]
                        nc.vector.tensor_copy(
                            dst.rearrange("p (ti b) -> p b ti", ti=4),
                            tp.rearrange("p (b ti) -> p b ti", b=BL))
                    zxr = RING.tile([128, CH * 4 * BL], bf16, tag="zx")
                    zx_tiles[k] = zxr
                    zview = zxr.rearrange("p (tl s b) -> p tl s b", s=4, b=BL)
                    nblk = CH * BL // 512
                    for s in range(4):
                        for jb in range(nblk):
                            xps = XPS.tile([128, 512], f32, tag="xp")
                            nc.tensor.matmul(
                                xps, K_bf[:, s * H:(s + 1) * H],
                                obsT[:, base + jb * 512: base + (jb + 1) * 512],
                                start=True, stop=False)
                            nc.tensor.matmul(
                                xps, b_bf[:, s * H:(s + 1) * H], ones_bf,
                                start=False, stop=True)
                            tpb = 512 // BL
                            dst = zview[:, jb * tpb:(jb + 1) * tpb, s, :]
                            nc.scalar.copy(
                                dst, xps.rearrange("p (tl b) -> p tl b", tl=tpb))

                def sweep1_pieces(blk):
                    R0 = blk * 512
                    st = {}
                    def p_post():
                        post = FPS.tile([128, 512], f32, tag="post",
                                        name="post")
                        st["post"] = post
                        nc.tensor.matmul(post, Wr_bf, Hs[:, R0:R0 + 512],
                                         start=True, stop=False)
                        nc.tensor.matmul(post, brS_bf, ones_bf, start=False,
                                         stop=True)
                    def p_cmblo():
                        nc.vector.tensor_copy(CMB[lo, R0:R0 + 512],
                                              st["post"][lo, :])
                    def p_cmbhi():
                        nc.scalar.activation(CMB[hi, R0:R0 + 512],
                                             st["post"][hi, :], FT.Exp)
                    def p_dm():
                        nc.vector.tensor_sub(Pst[lo, R0:R0 + 512],
                                             st["post"][lo, :],
                                             Pst[lo, R0:R0 + 512])
                    def p_esn():
                        nc.scalar.activation(Pst[hi, R0:R0 + 512],
                                             Pst[hi, R0:R0 + 512], FT.Exp)
                    def p_om():
                        om = FPS.tile([128, 512], f32, tag="gen", name="om")
                        st["om"] = om
                        nc.tensor.matmul(om, Wgm_bf, CMB[lo, R0:R0 + 512],
                                         start=True, stop=False)
                        nc.tensor.matmul(om, bgm_bf, ones_bf, start=False,
                                         stop=True)
                    def p_w():
                        nc.vector.tensor_sub(obsT[:, R0:R0 + 512],
                                             obsT[:, R0:R0 + 512], st["om"])
                    def p_os():
                        osp = FPS.tile([128, 512], f32, tag="gen", name="osp")
                        st["os"] = osp
                        nc.tensor.matmul(osp, Wgs_bf, CMB[lo, R0:R0 + 512],
                                         start=True, stop=False)
                        nc.tensor.matmul(osp, bgs_bf, ones_bf, start=False,
                                         stop=True)
                    def p_eos():
                        nc.scalar.activation(Hs[:, R0:R0 + 512], st["os"],
                                             FT.Exp)
                    return [p_post, p_cmblo, p_cmbhi, p_dm, p_esn,
                            p_om, p_w, p_os, p_eos]

                def chain_chunk(k, fillers=()):
                    nonlocal cprev, last_head
                    zxr = zx_tiles.pop(k)
                    zview = zxr
                    for tl in range(CH):
                        t = k * CH + tl
                        z = ZPS.tile([128, 4 * BL], f32, tag="z")
                        nc.tensor.matmul(z, idn_bf,
                                         zview[:, tl * 128:(tl + 1) * 128],
                                         start=True, stop=(t == 0),
                                         skip_group_check=True)
                        if t > 0:
                            for s in range(4):
                                nc.tensor.matmul(
                                    z[:, s * BL:(s + 1) * BL],
                                    R_bf[:, s * H:(s + 1) * H],
                                    Hs[:, (t - 1) * BL: t * BL],
                                    start=False, stop=(s == 3),
                                    skip_group_check=True)
                        tz = (tzA, tzB)[t % 2]
                        tznx = (tzA, tzB)[(t + 1) % 2]
                        nc.scalar.activation(tz[:, 0:4 * BL], z,
                                             FT.Tanh, scale=0.5)
                        # one stt: [q2|q1] = (tz[i|f]+1) * tz[g|C]
                        qq = CW.tile([128, 2 * BL], bf16, tag="qq")
                        nc.vector.scalar_tensor_tensor(
                            qq, tz[:, 0:2 * BL], one_c, tz[:, 3 * BL:5 * BL],
                            op0=ALU.add, op1=ALU.mult)
                        nc.vector.scalar_tensor_tensor(
                            tznx[:, 4 * BL:5 * BL], qq[:, BL:2 * BL], half_c,
                            qq[:, 0:BL], op0=ALU.mult, op1=ALU.add)
                        th = CW.tile([128, BL], bf16, tag="th")
                        nc.scalar.activation(th, tznx[:, 4 * BL:5 * BL],
                                             FT.Tanh, scale=0.5)
                        nc.vector.scalar_tensor_tensor(
                            Hs[:, t * BL:(t + 1) * BL], tz[:, 2 * BL:3 * BL],
                            one_c, th, op0=ALU.add, op1=ALU.mult)
                        if tl < len(fillers):
                            fillers[tl]()
                        if "P" in ABL:
                            continue
                        # prior chain step (independent recurrence)
                        pp = PPS.tile([128, BL], f32, tag="pp")
                        mprev = minit if t == 0 else Pst[0:S, (t - 1) * BL: t * BL]
                        nc.tensor.matmul(pp[0:2 * S, :], Wf_bf, mprev,
                                         start=True, stop=True)
                        nc.scalar.activation(Pst[:, t * BL:(t + 1) * BL],
                                             pp, FT.Identity, bias=pbias)

                cprev = c0
                last_head = None
                for k in range(NCHUNK + 1):
                    if k < NCHUNK and "A" not in ABL:
                        phase_a(k)
                    fill = []
                    if k >= 2:
                        fill = (sweep1_pieces(2 * (k - 2))
                                + sweep1_pieces(2 * (k - 2) + 1))
                    if k >= 1 and "C" not in ABL and "A" not in ABL:
                        chain_chunk(k - 1, fill)
                for blk in (2 * (NCHUNK - 1), 2 * (NCHUNK - 1) + 1):
                    for f in sweep1_pieces(blk):
                        f()

            # -------- sweep 2: log-domain reductions (one table switch) --
            with (
                tc.tile_pool(name="sps", bufs=2, space="PSUM") as SPS,
                tc.tile_pool(name="fw", bufs=2) as FW,
            ):
                # Ln ops must schedule after every chain/sweep-1 ACT op
                # (single table switch). Express as a data dependency: their
                # bias tiles derive from the last sweep-1 output column.
                gate0 = FW.tile([128, 1], f32, tag="g0")
                nc.vector.tensor_scalar_mul(gate0, Hs[:, TB - 1:TB], 0.0)
                gate1 = FW.tile([128, 1], f32, tag="g1")
                nc.vector.tensor_scalar_add(gate1, gate0, 1.0)

                def ln_act(out, in_, one, psl=slice(0, 128), **kw):
                    gate = gate1 if one else gate0
                    return nc.scalar.activation(out, in_, FT.Ln,
                                                bias=gate[psl, :], **kw)

                WB = 1024
                for blk in range(TB // WB if "F" not in ABL and "A" not in ABL else 0):
                    R0 = blk * WB
                    Rsl = slice(R0, R0 + WB)
                    W1 = FW.tile([128, WB], f32, tag="w1")
                    W2 = FW.tile([128, WB], f32, tag="w2")
                    W3 = FW.tile([128, WB], f32, tag="w3")
                    SPt = FW.tile([128, WB], f32, tag="sp")
                    SQt = FW.tile([128, WB], f32, tag="sq")
                    ln_act(SPt[hi, :], CMB[hi, Rsl], True, psl=hi)
                    ln_act(SQt[hi, :], Pst[hi, Rsl], True, psl=hi)
                    nc.vector.reciprocal(W1[hi, :], SPt[hi, :])
                    nc.vector.tensor_mul(W2[hi, :], SQt[hi, :], W1[hi, :])
                    ln_act(W3[hi, :], W2[hi, :], False, psl=hi,
                           accum_out=accsl(hi, 0, blk))
                    nc.vector.reciprocal(W3[hi, :], SQt[hi, :])
                    nc.vector.tensor_mul(W1[hi, :], SPt[hi, :], W3[hi, :])
                    nc.scalar.activation(W2[hi, :], W1[hi, :], FT.Square,
                                         accum_out=accsl(hi, 1, blk))
                    for j in range(WB // 512):
                        shf = SPS.tile([128, 512], f32, tag="shf")
                        nc.tensor.matmul(shf[lo, :], idn_f32[S:2 * S, S:2 * S],
                                         W3[hi, j * 512:(j + 1) * 512],
                                         start=True, stop=True)
                        nc.vector.tensor_mul(
                            W1[lo, j * 512:(j + 1) * 512],
                            Pst[lo, R0 + j * 512:R0 + (j + 1) * 512], shf[lo, :])
                    nc.scalar.activation(W2[lo, :], W1[lo, :], FT.Square,
                                         accum_out=accsl(lo, 2, blk))
                    # recon: os = log1p(exp), log os, ((obs-om)/os)^2
                    ln_act(W3, Hs[:, Rsl], True)
                    ln_act(W1, W3, False, accum_out=accsl(slice(0, 128), 3, blk))
                    nc.vector.reciprocal(W1, W3)
                    nc.vector.tensor_mul(W2, obsT[:, Rsl], W1)
                    nc.scalar.activation(W3, W2, FT.Square,
                                         accum_out=accsl(slice(0, 128), 4, blk))

                racc = FW.tile([128, 5], f32, tag="racc")
                nc.vector.tensor_reduce(
                    racc, acc.rearrange("p (k n) -> p k n", k=5),
                    axis=mybir.AxisListType.X, op=ALU.add)
                ones_col = FW.tile([128, 1], f32, tag="onec")
                nc.vector.memset(ones_col, 1.0)
                fin_ps = SPS.tile([1, 8], f32, tag="shf")
                nc.vector.memset(fin_ps, 0.0)
                nc.tensor.matmul(fin_ps[:, 0:5], ones_col, racc,
                                 start=True, stop=True)
                fin = FW.tile([1, 8], f32, tag="fin")
                nc.vector.tensor_copy(fin, fin_ps)
                nc.sync.dma_start(out=out_d.ap(), in_=fin)
    nc.compile()
    return nc


_CACHE = {}


def _get_nc(T=T_FULL, CH=32):
    key = (T, CH)
    if key not in _CACHE:
        _CACHE[key] = _build(T, CH)
    return _CACHE[key]


def _make_in_maps(inputs, ncores=NCORES):
    names = ["Wf_m", "bf_m", "Wf_s", "bf_s", "Wg_m", "bg_m", "Wg_s", "bg_s",
             "K_lstm", "R_lstm", "b_lstm", "Wr_m", "br_m", "Wr_s", "br_s"]
    shared = {n: np.ascontiguousarray(np.asarray(inputs[n], np.float32))
              for n in names}
    in_maps = []
    for c in range(ncores):
        sl = slice(c * BL, (c + 1) * BL)
        m = dict(shared)
        m["obs"] = np.ascontiguousarray(np.asarray(inputs["obs"], np.float32)[sl])
        m["initial_mean"] = np.ascontiguousarray(
            np.asarray(inputs["initial_mean"], np.float32)[sl])
        in_maps.append(m)
    return in_maps


def _assemble(accs, T=T_FULL):
    """accs: [ncores, 8] raw sums -> final scalar."""
    s = np.asarray(accs, np.float64).sum(axis=0)
    lnr, uu, vv, lnos, ww = s[0], 0.5 * s[1], 0.5 * s[2], s[3], 0.5 * s[4]
    kl_total = lnr + uu + vv - 0.5 * B_GLOBAL * T * S
    mean_kl = kl_total / (B_GLOBAL * T)
    mean_recon = (ww + lnos + 0.5 * LOG2PI * B_GLOBAL * T * O) / B_GLOBAL
    return np.float32(mean_kl + mean_recon)


def kernel(**inputs):
    from concourse.bass_utils import run_bass_kernel_spmd
    nc = _get_nc()
    in_maps = _make_in_maps(inputs)
    res = run_bass_kernel_spmd(nc, in_maps, core_ids=list(range(NCORES)))
    accs = [r["out"][0] for r in res.results]
    return _assemble(accs)

